# revision 9
# baseline (speedup 1.0000x reference)
"""PINN (IRK tanh-MLP + u_xx) Trainium2 kernel — grid-interpolation form.

Every activation of this network is a smooth function of the single scalar
input x, so the map x -> (U0, U1) rows is 100 smooth 1-D functions.  The
device evaluates the MLP once on a fixed 128-node uniform grid covering
[-5.5, 5.4], forms F = -(5u - 5u^3 + 5e-4*u_xx) at the nodes (u_xx via an
exact-cancellation 3-point FD in fp32), folds the IRK matrices into two
128x100 node "combo" matrices C0/C1 with one tiny matmul each, and then
produces U0/U1 for all 8192 collocation points of the core with a single
fp16 matmul  C^T @ M,  where M is the host-built (data-layout) matrix of
cubic-Lagrange interpolation weights: 4 nonzeros per column, dense
(128 x 8192) fp16.  Cubic interpolation on this grid reproduces the exact
network outputs to ~1e-6; fp16 rounding brings the end-to-end error to
~1e-3, well inside the 2e-2 gate.  Data-parallel over 8 cores (x batch-
sharded, weights replicated).  Power-of-2 scales (FS=256 on F, CS=8 on
C0/C1) keep fp16 magnitudes in range; the host multiplies outputs by CS.

Schedule notes: the tanh activation table is preloaded at t=0; constants
arrive in two DMAs (early layers first) so the grid eval starts ~1.7 us
while the interpolation matrix streams in behind it; the 16-tile main loop
spreads its PSUM->SBUF fp16 casts round-robin over Act/Pool/DVE; outputs
leave in 5 staggered group DMAs on the SP queue.
"""

import sys

sys.path.insert(0, "/opt/trn_rl_repo")

import numpy as np

import concourse.bass as bass
import concourse.mybir as mybir
import concourse.tile as tile
from concourse import bacc
from concourse.masks import make_identity

F32 = mybir.dt.float32
FP16 = mybir.dt.float16
AF = mybir.ActivationFunctionType
ALU = mybir.AluOpType

N_CORES = 8
N_TOTAL = 65536
NC = N_TOTAL // N_CORES  # 8192 points per core
TILE = 512
T = NC // TILE           # 16 tiles
Q = 100
DT = 0.8
LAYERS = [1, 20, 50, 200, 500, 200, Q]

G = 128                  # grid nodes (one PE partition block)
G0 = -5.5
DLT = 11.0 / 128.0       # grid spacing; nodes exactly representable in fp16
FDC = 1e-4 / (DLT * DLT)
FS = 256.0               # F-node scale (keeps u^3 inside fp16 range)
CS = 8.0                 # combo scale (outputs are U/CS; host multiplies back)


def _chunks(n):
    out = []
    s = 0
    while s < n:
        sz = min(128, n - s)
        out.append((s, sz))
        s += sz
    return out


# wk16a: early constants (layer 0-2 weights + rows)
OFF_WT1 = 0                    # [128, 50]   rows 0:20
OFF_WT2 = OFF_WT1 + 50         # [128, 200]  rows 0:50
OFF_ONES = OFF_WT2 + 200       # [128, 128]  row 0 = 1.0
OFF_GX = OFF_ONES + 128        # [128, 128]  row 0 = grid x (fp16-exact)
OFF_XSQ = OFF_GX + 128         # [128, 128]  row 0 = gx^2 - 1
C16A = OFF_XSQ + 128
# wk16b: late constants (layer 3-5 weights + IRK combos)
OFF_WT3 = 0                    # [128, 1000] 2 k-chunks of 500
OFF_WT4 = OFF_WT3 + 1000       # [128, 800]  4 k-chunks of 200
OFF_WT5 = OFF_WT4 + 800        # [128, 200]  2 k-chunks of 100
OFF_G1 = OFF_WT5 + 200         # [128, 100]  rows 0:100
OFF_G2 = OFF_G1 + 100          # [128, 100]
C16B = OFF_G2 + 100

# wk32 fp32 column offsets
O32_W0 = 0   # rows 0:20 = W0[:,0]
O32_B0 = 1   # rows 0:20 = b0
O32_B1 = 2   # rows 0:50 = b1
O32_B2 = 3   # 2 cols
O32_B3 = 5   # 4 cols
O32_B4 = 9   # 2 cols
O32_B5 = 11  # rows 0:100 = b5
C32 = 12

# output DMA groups (in tiles): staggered, small final group for short tail.
# The staging/DRAM layout interleaves U0/U1 by group:
#   [g0:U0 | g0:U1 | g1:U0 | g1:U1 | ...]  so each group is ONE linear DMA.
GROUPS = [(0, 6), (6, 4), (10, 4), (14, 2)]
GBASE = {}
_acc = 0
for _g0, _gn in GROUPS:
    GBASE[_g0] = _acc
    _acc += 2 * _gn * TILE


def build_kernel(reps=1):
    nc = bacc.Bacc("TRN2", target_bir_lowering=False, debug=False,
                   num_devices=N_CORES)

    wk16a_e = nc.declare_dram_parameter("wk16a", [128, C16A], FP16,
                                        isOutput=False)
    wk16b_e = nc.declare_dram_parameter("wk16b", [128, C16B], FP16,
                                        isOutput=False)
    wk32_e = nc.declare_dram_parameter("wk32", [128, C32], F32,
                                       isOutput=False)
    msb_e = nc.declare_dram_parameter("msb", [128, NC], FP16, isOutput=False)
    u01_e = nc.declare_dram_parameter("U01", [Q, 2 * NC], FP16,
                                      isOutput=True)

    from contextlib import ExitStack
    with tile.TileContext(nc) as tc, ExitStack() as es:
        wpool = es.enter_context(tc.tile_pool(name="weights", bufs=1))
        npool = es.enter_context(tc.tile_pool(name="nodes", bufs=1))
        pgrid = es.enter_context(tc.tile_pool(name="pgrid", bufs=2,
                                              space="PSUM"))
        pmain = es.enter_context(tc.tile_pool(name="pmain", bufs=2,
                                              space="PSUM"))

        # ---- t=0: preload tanh activation table (off critical path) -----
        scr = npool.tile([1, 2], F32, name="scr")
        nc.vector.memset(scr[0:1, 0:1], 0.0)
        nc.scalar.activation(scr[0:1, 1:2], scr[0:1, 0:1], AF.Tanh)

        # identity for PE transposes — BEFORE the DMAs in the Pool queue
        identh = wpool.tile([128, 128], FP16, name="identh")
        make_identity(nc, identh[:, :])

        # ---- input DMAs (gpsimd/Pool queue, earliest-needed first) ------
        wk16a = wpool.tile([128, C16A], FP16, name="wk16a_sb")
        nc.gpsimd.dma_start(out=wk16a[:, :], in_=wk16a_e[:, :])
        wk32 = wpool.tile([128, C32], F32, name="wk32_sb")
        nc.gpsimd.dma_start(out=wk32[:, :], in_=wk32_e[:, :])
        wk16b = wpool.tile([128, C16B], FP16, name="wk16b_sb")
        nc.gpsimd.dma_start(out=wk16b[:, :], in_=wk16b_e[:, :])
        msb = wpool.tile([128, NC], FP16, name="msb_sb")
        HALF = NC // 2
        nc.gpsimd.dma_start(out=msb[:, 0:HALF], in_=msb_e[:, 0:HALF])
        nc.gpsimd.dma_start(out=msb[:, HALF:NC], in_=msb_e[:, HALF:NC])

        # ---- grid MLP eval (batch = 128 grid nodes, feature-major) ------
        ph0 = pgrid.tile([128, G], F32, name="ph0", tag="pg")
        nc.tensor.matmul(ph0[0:20, :], wk16a[0:1, OFF_ONES:OFF_ONES + 20],
                         wk16a[0:1, OFF_GX:OFF_GX + G], start=True, stop=True)
        # broadcast (gx^2 - 1) along partitions (needs only wk16a)
        pxsq = pgrid.tile([128, G], F32, name="pxsq", tag="px", bufs=1)
        nc.tensor.matmul(pxsq[0:Q, :], wk16a[0:1, OFF_ONES:OFF_ONES + Q],
                         wk16a[0:1, OFF_XSQ:OFF_XSQ + G], start=True,
                         stop=True)
        h0 = npool.tile([128, G], FP16, name="h0")
        nc.scalar.activation(h0[0:20, :], ph0[0:20, :], AF.Tanh,
                             bias=wk32[0:20, O32_B0:O32_B0 + 1],
                             scale=wk32[0:20, O32_W0:O32_W0 + 1])

        wsrc = {1: (None, OFF_WT1), 2: (None, OFF_WT2),
                3: (True, OFF_WT3), 4: (True, OFF_WT4)}
        bc_off = {1: O32_B1, 2: O32_B2, 3: O32_B3, 4: O32_B4}
        prev_h = h0
        for l in range(1, 5):
            fi, fo = LAYERS[l], LAYERS[l + 1]
            kcs = _chunks(fi)
            mcs = _chunks(fo)
            wk = wk16b if wsrc[l][0] else wk16a
            off = wsrc[l][1]
            h_n = npool.tile([128, len(mcs) * G], FP16, name=f"h{l}")
            if l == 4:
                # rows 72:96 zero, row 96 = 1.0: the L5 bias row (b5 folded
                # into wt5b at k-row 96 of chunk 1)
                nc.vector.memset(h_n[64:96, G:2 * G], 0.0)
                nc.vector.memset(h_n[96:97, G:2 * G], 1.0)
            for mi, (mo, ms) in enumerate(mcs):
                ph = pgrid.tile([128, G], F32, name=f"ph{l}_{mi}", tag="pg")
                for ki, (ko, ks) in enumerate(kcs):
                    nc.tensor.matmul(
                        ph[0:ms, :],
                        wk[0:ks, off + ki * fo + mo:off + ki * fo + mo + ms],
                        prev_h[0:ks, ki * G:(ki + 1) * G],
                        start=(ki == 0), stop=(ki == len(kcs) - 1))
                nc.scalar.activation(
                    h_n[0:ms, mi * G:(mi + 1) * G], ph[0:ms, :], AF.Tanh,
                    bias=wk32[0:ms, bc_off[l] + mi:bc_off[l] + mi + 1])
            prev_h = h_n

        # layer 5: fi=200+bias row (chunks of 128 and 97), out (100, G)
        pL5 = pgrid.tile([128, G], F32, name="pL5", tag="pg")
        nc.tensor.matmul(pL5[0:Q, :], wk16b[0:128, OFF_WT5:OFF_WT5 + Q],
                         prev_h[0:128, 0:G], start=True, stop=False)
        nc.tensor.matmul(pL5[0:Q, :],
                         wk16b[0:97, OFF_WT5 + Q:OFF_WT5 + 2 * Q],
                         prev_h[0:97, G:2 * G], start=False, stop=True)

        # ---- node-side math (all [100, 128] fp32, trivial sizes) --------
        # u = pxsq * pL5 - 1   (b5 already folded into the L5 matmul)
        u = npool.tile([128, G], F32, name="u_fm")
        nc.vector.tensor_mul(u[0:Q, :], pxsq[0:Q, :], pL5[0:Q, :])
        nc.vector.tensor_scalar_add(u[0:Q, :], u[0:Q, :], -1.0)

        # wfd = u[i-1] + u[i+1] - 2 u[i]  (grid-axis FD; edge cols zero)
        wfd = npool.tile([128, G], F32, name="wfd")
        nc.vector.memset(wfd[0:Q, 0:1], 0.0)
        nc.vector.memset(wfd[0:Q, G - 1:G], 0.0)
        z = npool.tile([128, G], F32, name="z")
        nc.vector.tensor_add(z[0:Q, 1:G - 1], u[0:Q, 0:G - 2], u[0:Q, 2:G])
        nc.vector.scalar_tensor_tensor(wfd[0:Q, 1:G - 1], u[0:Q, 1:G - 1],
                                       -2.0, z[0:Q, 1:G - 1], ALU.mult,
                                       ALU.add)

        # Fn = (5/FS)*(u^3 - u) - (5*FDC/FS)*wfd
        usq = npool.tile([128, G], F32, name="usq")
        nc.vector.tensor_mul(usq[0:Q, :], u[0:Q, :], u[0:Q, :])
        nc.vector.tensor_scalar_add(usq[0:Q, :], usq[0:Q, :], -1.0)
        gs = npool.tile([128, G], F32, name="gs")
        nc.vector.scalar_tensor_tensor(gs[0:Q, :], u[0:Q, :], 5.0 / FS,
                                       usq[0:Q, :], ALU.mult, ALU.mult)
        fn16 = npool.tile([128, G], FP16, name="fn16")
        nc.vector.scalar_tensor_tensor(fn16[0:Q, :], wfd[0:Q, :],
                                       -5.0 * FDC / FS, gs[0:Q, :], ALU.mult,
                                       ALU.add)

        # ---- combo matrices: C = u/CS + G' @ Fn -------------------------
        lt = npool.tile([128, 256], FP16, name="lt")
        nc.vector.memset(lt[:, 100:128], 0.0)
        nc.vector.memset(lt[:, 228:256], 0.0)
        for which, goff, lcol in ((0, OFF_G1, 0), (1, OFF_G2, 128)):
            pc = pgrid.tile([128, G], F32, name=f"pc{which}", tag="pg")
            nc.tensor.matmul(pc[0:Q, :], wk16b[0:Q, goff:goff + Q],
                             fn16[0:Q, :], start=True, stop=True)
            c16 = npool.tile([128, G], FP16, name=f"c16_{which}")
            nc.vector.scalar_tensor_tensor(c16[0:Q, :], u[0:Q, :], 1.0 / CS,
                                           pc[0:Q, :], ALU.mult, ALU.add)
            ptr = pgrid.tile([128, G], FP16, name=f"ptr{which}", tag="pt",
                             bufs=1)
            nc.tensor.transpose(ptr[0:G, 0:Q], c16[0:Q, 0:G],
                                identh[0:Q, 0:Q])
            nc.vector.tensor_copy(lt[:, lcol:lcol + Q], ptr[0:G, 0:Q])

        # ---- main interpolation loop ------------------------------------
        # PSUM->SBUF fp16 casts spread over Pool/Act/DVE weighted by their
        # per-op cost (427/612/658 ns); matmuls paired per lhsT to halve
        # Ldweights reloads.  Staging tile ou01 is group-interleaved.
        pool_cp = nc.gpsimd.tensor_copy
        act_cp = nc.scalar.copy
        dve_cp = nc.vector.tensor_copy
        casters = [pool_cp, act_cp, dve_cp,
                   pool_cp, dve_cp, act_cp] * 6  # ~ balanced rotation
        ou01 = wpool.tile([128, 2 * NC], FP16, name="ou01")

        def odst(t, which):
            for g0t, gn in GROUPS:
                if g0t <= t < g0t + gn:
                    base = GBASE[g0t] + which * gn * TILE + (t - g0t) * TILE
                    return slice(base, base + TILE)

        for _rep in range(reps):
            ci = 0
            for tp in range(0, T, 2):
                pas, pbs = [], []
                for t in (tp, tp + 1):
                    sl = slice(t * TILE, (t + 1) * TILE)
                    pa = pmain.tile([128, TILE], F32, name=f"pa{t}",
                                    tag="pa")
                    nc.tensor.matmul(pa[:, :], lt[:, 0:128], msb[:, sl],
                                     start=True, stop=True)
                    pas.append(pa)
                for t in (tp, tp + 1):
                    sl = slice(t * TILE, (t + 1) * TILE)
                    pb = pmain.tile([128, TILE], F32, name=f"pb{t}",
                                    tag="pb")
                    nc.tensor.matmul(pb[:, :], lt[:, 128:256], msb[:, sl],
                                     start=True, stop=True)
                    pbs.append(pb)
                for t, pa, pb in zip((tp, tp + 1), pas, pbs):
                    casters[ci](ou01[0:Q, odst(t, 0)], pa[0:Q, :])
                    casters[ci + 1](ou01[0:Q, odst(t, 1)], pb[0:Q, :])
                    ci += 2
                for g0t, gn in GROUPS:
                    if tp + 1 == g0t + gn - 1:
                        gs_ = slice(GBASE[g0t], GBASE[g0t] + 2 * gn * TILE)
                        nc.sync.dma_start(out=u01_e[0:Q, gs_],
                                          in_=ou01[0:Q, gs_])

    nc.compile()
    return nc


def prep_inputs(W, b, x, A, bvec):
    """Host-side prep: packed replicated constants + per-core M matrices."""
    wk16a = np.zeros((128, C16A), np.float32)
    wk16a[0:20, OFF_WT1:OFF_WT1 + 50] = W[1].T
    wk16a[0:50, OFF_WT2:OFF_WT2 + 200] = W[2].T
    wk16a[0, OFF_ONES:OFF_ONES + 128] = 1.0
    gx = (G0 + DLT * np.arange(G)).astype(np.float32)
    gx16 = gx.astype(np.float16).astype(np.float32)
    wk16a[0, OFF_GX:OFF_GX + G] = gx16
    wk16a[0, OFF_XSQ:OFF_XSQ + G] = gx16 * gx16 - 1.0

    wk16b = np.zeros((128, C16B), np.float32)
    for l, off in ((3, OFF_WT3), (4, OFF_WT4)):
        fi, fo = LAYERS[l], LAYERS[l + 1]
        for ki, (ko, ks) in enumerate(_chunks(fi)):
            wk16b[0:ks, off + ki * fo:off + (ki + 1) * fo] = \
                W[l].T[ko:ko + ks, :]
    wk16b[0:128, OFF_WT5:OFF_WT5 + Q] = W[5].T[0:128, :]
    wk16b[0:72, OFF_WT5 + Q:OFF_WT5 + 2 * Q] = W[5].T[128:200, :]
    wk16b[96, OFF_WT5 + Q:OFF_WT5 + 2 * Q] = b[5]
    cg = DT * FS / CS
    wk16b[0:Q, OFF_G1:OFF_G1 + Q] = cg * A.T
    wk16b[0:Q, OFF_G2:OFF_G2 + Q] = cg * (A - np.ones((Q, 1)) @ bvec).T

    wk32 = np.zeros((128, C32), np.float32)
    wk32[0:20, O32_W0] = W[0][:, 0]
    wk32[0:20, O32_B0] = b[0]
    wk32[0:50, O32_B1] = b[1]
    for l, off in ((2, O32_B2), (3, O32_B3), (4, O32_B4)):
        for mi, (mo, ms) in enumerate(_chunks(LAYERS[l + 1])):
            wk32[0:ms, off + mi] = b[l][mo:mo + ms]
    wk32[0:Q, O32_B5] = b[5]

    common = {"wk16a": wk16a.astype(np.float16),
              "wk16b": wk16b.astype(np.float16), "wk32": wk32}

    xf = np.asarray(x, np.float64).reshape(-1)
    s = (xf - G0) / DLT
    iv = np.clip(np.floor(s).astype(np.int64), 1, G - 3)
    t = s - iv
    w4 = np.stack([-t * (t - 1) * (t - 2) / 6.0,
                   (t + 1) * (t - 1) * (t - 2) / 2.0,
                   -(t + 1) * t * (t - 2) / 2.0,
                   (t + 1) * t * (t - 1) / 6.0], axis=0)  # (4, N)
    M = np.zeros((G, N_TOTAL), np.float32)
    cols = np.arange(N_TOTAL)
    for j in range(4):
        M[iv + j - 1, cols] = w4[j]
    M = M.astype(np.float16)
    shards = [{"msb": M[:, c * NC:(c + 1) * NC]} for c in range(N_CORES)]
    return common, shards


_NC_CACHE = None


def kernel(W0, b0, W1, b1, W2, b2, W3, b3, W4, b4, W5, b5, x, A, bvec):
    global _NC_CACHE
    W = [np.asarray(w, np.float32) for w in (W0, W1, W2, W3, W4, W5)]
    bs = [np.asarray(v, np.float32) for v in (b0, b1, b2, b3, b4, b5)]
    x = np.asarray(x, np.float32)
    A = np.asarray(A, np.float32)
    bvec = np.asarray(bvec, np.float32)

    if _NC_CACHE is None:
        _NC_CACHE = build_kernel()
    nc = _NC_CACHE

    common, shards = prep_inputs(W, bs, x, A, bvec)
    in_maps = [{**common, **shards[c]} for c in range(N_CORES)]

    from concourse.bass_utils import run_bass_kernel_spmd
    res = run_bass_kernel_spmd(nc, in_maps, list(range(N_CORES)))
    U0 = np.concatenate([deinterleave(res.results[c]["U01"], 0)
                         for c in range(N_CORES)], 0)
    U1 = np.concatenate([deinterleave(res.results[c]["U01"], 1)
                         for c in range(N_CORES)], 0)
    return U0, U1


def deinterleave(u01, which):
    """Recover U-part `which` (NC, Q) fp32 from the group-interleaved
    (Q, 2*NC) fp16 device output."""
    parts = []
    for g0t, gn in GROUPS:
        base = GBASE[g0t] + which * gn * TILE
        parts.append(u01[:, base:base + gn * TILE])
    return (np.concatenate(parts, 1).astype(np.float32).T * CS)


# revision 10
# speedup vs baseline: 1.2889x; 1.2889x over previous
"""PINN (IRK tanh-MLP + u_xx) Trainium2 kernel — grid-interpolation form.

Every activation of this network is a smooth function of the single scalar
input x, so the map x -> (U0, U1) rows is 100 smooth 1-D functions.  The
device evaluates the MLP once on a fixed 64-node uniform grid covering
[-5.5, 5.33], forms F = -(5u - 5u^3 + 5e-4*u_xx) at the nodes (u_xx via an
exact-cancellation 3-point FD in fp32), folds the IRK matrix A into a
64x101 node "combo" matrix  C = [u/CS + (DT*A.T/CS) @ F ; (DT/CS)*bvec @ F]
with one tiny matmul, and produces all outputs for the core's 8192
collocation points with a single fp16 matmul  C^T @ M,  where M is the
host-built (data-layout-only) matrix of cubic-Lagrange interpolation
weights: 4 nonzeros per column, dense (64 x 8192) fp16.  Row 100 of the
result is d = DT*(F @ bvec.T);  U0 = rows 0:100,  U1 = U0 - d (host-side
subtract of the broadcast row, as in the reference).  Cubic interpolation
on this grid reproduces the exact network outputs to ~1e-5; fp16 rounding
sets the end-to-end error at ~1e-3, well inside the 2e-2 gate.
Data-parallel over 8 cores (x batch-sharded, weights replicated).
Power-of-2 scales (FS=256 on F, CS=8 on C) keep fp16 in range; the host
multiplies outputs by CS.

Schedule notes: tanh activation table preloaded at t=0; constants arrive
in three DMAs (early layers first) so the grid eval starts ~2.5 us while
the interpolation matrix streams in behind it; the 16-tile main loop is a
single matmul + one PSUM->SBUF fp16 cast per tile, casts rotating over
Pool/Act/DVE; outputs leave in 4 staggered group DMAs on the SP queue.
"""

import sys

sys.path.insert(0, "/opt/trn_rl_repo")

import numpy as np

import concourse.bass as bass
import concourse.mybir as mybir
import concourse.tile as tile
from concourse import bacc
from concourse.masks import make_identity

F32 = mybir.dt.float32
FP16 = mybir.dt.float16
AF = mybir.ActivationFunctionType
ALU = mybir.AluOpType

N_CORES = 8
N_TOTAL = 65536
NC = N_TOTAL // N_CORES  # 8192 points per core
TILE = 512
T = NC // TILE           # 16 tiles
Q = 100
DT = 0.8
LAYERS = [1, 20, 50, 200, 500, 200, Q]

G = 64                   # grid nodes
G0 = -5.5
DLT = 11.0 / 64.0        # grid spacing; nodes exactly representable in fp16
FDC = 1e-4 / (DLT * DLT)
FS = 256.0               # F-node scale (keeps u^3 inside fp16 range)
CS = 8.0                 # combo scale (outputs are U/CS; host multiplies back)


def _chunks(n):
    out = []
    s = 0
    while s < n:
        sz = min(128, n - s)
        out.append((s, sz))
        s += sz
    return out


# wk16a: early constants (layer 0-2 weights + broadcast rows)
OFF_WT1 = 0                    # [128, 50]   rows 0:20
OFF_WT2 = OFF_WT1 + 50         # [128, 200]  rows 0:50
OFF_ONES = OFF_WT2 + 200       # [128, 100]  row 0 = 1.0
OFF_GX = OFF_ONES + 100        # [128, 64]   row 0 = grid x (fp16-exact)
OFF_XSQ = OFF_GX + G           # [128, 64]   row 0 = gx^2 - 1
C16A = OFF_XSQ + G
# wk16b: late constants (layer 3-5 weights + IRK combo with bvec row)
OFF_WT3 = 0                    # [128, 1000] 2 k-chunks of 500
OFF_WT4 = OFF_WT3 + 1000       # [128, 800]  4 k-chunks of 200
OFF_WT5 = OFF_WT4 + 800        # [128, 200]  2 k-chunks of 100 (b5 at k-row 96)
OFF_G1 = OFF_WT5 + 200         # [128, 101]  rows 0:100; col 100 = bvec row
C16B = OFF_G1 + Q + 1

# wk32 fp32 column offsets
O32_W0 = 0   # rows 0:20 = W0[:,0]
O32_B0 = 1   # rows 0:20 = b0
O32_B1 = 2   # rows 0:50 = b1
O32_B2 = 3   # 2 cols
O32_B3 = 5   # 4 cols
O32_B4 = 9   # 2 cols
C32 = 11

# output DMA groups (in tiles): staggered, small final group for short tail
GROUPS = [(0, 6), (6, 4), (10, 4), (14, 2)]


def build_kernel(reps=1):
    nc = bacc.Bacc("TRN2", target_bir_lowering=False, debug=False,
                   num_devices=N_CORES)

    wk16a_e = nc.declare_dram_parameter("wk16a", [128, C16A], FP16,
                                        isOutput=False)
    wk16b_e = nc.declare_dram_parameter("wk16b", [128, C16B], FP16,
                                        isOutput=False)
    wk32_e = nc.declare_dram_parameter("wk32", [128, C32], F32,
                                       isOutput=False)
    msb_e = nc.declare_dram_parameter("msb", [G, NC], FP16, isOutput=False)
    u0d_e = nc.declare_dram_parameter("U0d", [Q + 1, NC], FP16,
                                      isOutput=True)

    from contextlib import ExitStack
    with tile.TileContext(nc) as tc, ExitStack() as es:
        wpool = es.enter_context(tc.tile_pool(name="weights", bufs=1))
        npool = es.enter_context(tc.tile_pool(name="nodes", bufs=1))
        pgrid = es.enter_context(tc.tile_pool(name="pgrid", bufs=2,
                                              space="PSUM"))
        pmain = es.enter_context(tc.tile_pool(name="pmain", bufs=3,
                                              space="PSUM"))

        # ---- t=0: preload tanh activation table (off critical path) -----
        scr = npool.tile([1, 2], F32, name="scr")
        nc.vector.memset(scr[0:1, 0:1], 0.0)
        nc.scalar.activation(scr[0:1, 1:2], scr[0:1, 0:1], AF.Tanh)

        # identity for the PE transpose — BEFORE the DMAs in the Pool queue
        identh = wpool.tile([128, 128], FP16, name="identh")
        make_identity(nc, identh[:, :])

        # ---- input DMAs (gpsimd/Pool queue, earliest-needed first) ------
        wk16a = wpool.tile([128, C16A], FP16, name="wk16a_sb")
        nc.gpsimd.dma_start(out=wk16a[:, :], in_=wk16a_e[:, :])
        wk32 = wpool.tile([128, C32], F32, name="wk32_sb")
        nc.gpsimd.dma_start(out=wk32[:, :], in_=wk32_e[:, :])
        wk16b = wpool.tile([128, C16B], FP16, name="wk16b_sb")
        nc.gpsimd.dma_start(out=wk16b[:, :], in_=wk16b_e[:, :])
        msb = wpool.tile([G, NC], FP16, name="msb_sb")
        nc.gpsimd.dma_start(out=msb[:, :], in_=msb_e[:, :])

        # ---- grid MLP eval (batch = 64 grid nodes, feature-major) -------
        ph0 = pgrid.tile([128, G], F32, name="ph0", tag="pg")
        nc.tensor.matmul(ph0[0:20, :], wk16a[0:1, OFF_ONES:OFF_ONES + 20],
                         wk16a[0:1, OFF_GX:OFF_GX + G], start=True, stop=True)
        # broadcast (gx^2 - 1) along partitions (needs only wk16a)
        pxsq = pgrid.tile([128, G], F32, name="pxsq", tag="px", bufs=1)
        nc.tensor.matmul(pxsq[0:Q, :], wk16a[0:1, OFF_ONES:OFF_ONES + Q],
                         wk16a[0:1, OFF_XSQ:OFF_XSQ + G], start=True,
                         stop=True)
        h0 = npool.tile([128, G], FP16, name="h0")
        nc.scalar.activation(h0[0:20, :], ph0[0:20, :], AF.Tanh,
                             bias=wk32[0:20, O32_B0:O32_B0 + 1],
                             scale=wk32[0:20, O32_W0:O32_W0 + 1])

        wsrc = {1: (None, OFF_WT1), 2: (None, OFF_WT2),
                3: (True, OFF_WT3), 4: (True, OFF_WT4)}
        bc_off = {1: O32_B1, 2: O32_B2, 3: O32_B3, 4: O32_B4}
        prev_h = h0
        for l in range(1, 5):
            fi, fo = LAYERS[l], LAYERS[l + 1]
            kcs = _chunks(fi)
            mcs = _chunks(fo)
            wk = wk16b if wsrc[l][0] else wk16a
            off = wsrc[l][1]
            h_n = npool.tile([128, len(mcs) * G], FP16, name=f"h{l}")
            if l == 4:
                # rows 72:96 zero, row 96 = 1.0 in chunk 1: the L5 bias row
                # (b5 folded into wt5b at k-row 96)
                nc.vector.memset(h_n[64:96, G:2 * G], 0.0)
                nc.vector.memset(h_n[96:97, G:2 * G], 1.0)
            for mi, (mo, ms) in enumerate(mcs):
                ph = pgrid.tile([128, G], F32, name=f"ph{l}_{mi}", tag="pg")
                for ki, (ko, ks) in enumerate(kcs):
                    nc.tensor.matmul(
                        ph[0:ms, :],
                        wk[0:ks, off + ki * fo + mo:off + ki * fo + mo + ms],
                        prev_h[0:ks, ki * G:(ki + 1) * G],
                        start=(ki == 0), stop=(ki == len(kcs) - 1))
                nc.scalar.activation(
                    h_n[0:ms, mi * G:(mi + 1) * G], ph[0:ms, :], AF.Tanh,
                    bias=wk32[0:ms, bc_off[l] + mi:bc_off[l] + mi + 1])
            prev_h = h_n

        # layer 5: fi=200 + bias row (chunks of 128 and 97), out (100, G)
        pL5 = pgrid.tile([128, G], F32, name="pL5", tag="pg")
        nc.tensor.matmul(pL5[0:Q, :], wk16b[0:128, OFF_WT5:OFF_WT5 + Q],
                         prev_h[0:128, 0:G], start=True, stop=False)
        nc.tensor.matmul(pL5[0:Q, :],
                         wk16b[0:97, OFF_WT5 + Q:OFF_WT5 + 2 * Q],
                         prev_h[0:97, G:2 * G], start=False, stop=True)

        # ---- node-side math (all [100, 64] fp32, trivial sizes) ---------
        # u = pxsq * pL5 - 1     (rows 96:128 zeroed so combo row 100 = d)
        u = npool.tile([128, G], F32, name="u_fm")
        nc.vector.memset(u[96:128, :], 0.0)
        nc.vector.tensor_mul(u[0:Q, :], pxsq[0:Q, :], pL5[0:Q, :])
        nc.vector.tensor_scalar_add(u[0:Q, :], u[0:Q, :], -1.0)

        # wfd = u[i-1] + u[i+1] - 2 u[i]  (grid-axis FD; edge cols zero)
        wfd = npool.tile([128, G], F32, name="wfd")
        nc.vector.memset(wfd[0:Q, 0:1], 0.0)
        nc.vector.memset(wfd[0:Q, G - 1:G], 0.0)
        z = npool.tile([128, G], F32, name="z")
        nc.vector.tensor_add(z[0:Q, 1:G - 1], u[0:Q, 0:G - 2], u[0:Q, 2:G])
        nc.vector.scalar_tensor_tensor(wfd[0:Q, 1:G - 1], u[0:Q, 1:G - 1],
                                       -2.0, z[0:Q, 1:G - 1], ALU.mult,
                                       ALU.add)

        # Fn = (5/FS)*(u^3 - u) - (5*FDC/FS)*wfd
        usq = npool.tile([128, G], F32, name="usq")
        nc.vector.tensor_mul(usq[0:Q, :], u[0:Q, :], u[0:Q, :])
        nc.vector.tensor_scalar_add(usq[0:Q, :], usq[0:Q, :], -1.0)
        gs = npool.tile([128, G], F32, name="gs")
        nc.vector.scalar_tensor_tensor(gs[0:Q, :], u[0:Q, :], 5.0 / FS,
                                       usq[0:Q, :], ALU.mult, ALU.mult)
        fn16 = npool.tile([128, G], FP16, name="fn16")
        nc.vector.scalar_tensor_tensor(fn16[0:Q, :], wfd[0:Q, :],
                                       -5.0 * FDC / FS, gs[0:Q, :], ALU.mult,
                                       ALU.add)

        # ---- combo: C[0:100] = u/CS + G1' @ Fn ; C[100] = bvec' @ Fn ----
        pc = pgrid.tile([128, G], F32, name="pc", tag="pg")
        nc.tensor.matmul(pc[0:Q + 1, :], wk16b[0:Q, OFF_G1:OFF_G1 + Q + 1],
                         fn16[0:Q, :], start=True, stop=True)
        c16 = npool.tile([128, G], FP16, name="c16")
        nc.vector.scalar_tensor_tensor(c16[0:Q + 1, :], u[0:Q + 1, :],
                                       1.0 / CS, pc[0:Q + 1, :], ALU.mult,
                                       ALU.add)
        ptr = pgrid.tile([128, 128], FP16, name="ptr", tag="pt", bufs=1)
        nc.tensor.transpose(ptr[0:G, 0:Q + 1], c16[0:Q + 1, 0:G],
                            identh[0:Q + 1, 0:Q + 1])
        lt = npool.tile([G, 128], FP16, name="lt")
        nc.vector.memset(lt[:, 96:128], 0.0)
        nc.vector.tensor_copy(lt[:, 0:Q + 1], ptr[0:G, 0:Q + 1])

        # ---- main interpolation loop: 1 matmul + 1 cast per tile --------
        casters = [nc.gpsimd.tensor_copy, nc.scalar.copy,
                   nc.vector.tensor_copy]
        ou = wpool.tile([128, NC], FP16, name="ou")
        for _rep in range(reps):
            for t in range(T):
                sl = slice(t * TILE, (t + 1) * TILE)
                pa = pmain.tile([128, TILE], F32, name=f"pa{t}", tag="pa")
                nc.tensor.matmul(pa[:, :], lt[0:G, 0:128], msb[0:G, sl],
                                 start=True, stop=True)
                casters[t % 3](ou[0:Q + 1, sl], pa[0:Q + 1, :])
                for g0t, gn in GROUPS:
                    if t == g0t + gn - 1:
                        gs_ = slice(g0t * TILE, (g0t + gn) * TILE)
                        nc.sync.dma_start(out=u0d_e[0:Q + 1, gs_],
                                          in_=ou[0:Q + 1, gs_])

    nc.compile()
    return nc


def prep_inputs(W, b, x, A, bvec):
    """Host-side prep: packed replicated constants + per-core M matrices."""
    wk16a = np.zeros((128, C16A), np.float32)
    wk16a[0:20, OFF_WT1:OFF_WT1 + 50] = W[1].T
    wk16a[0:50, OFF_WT2:OFF_WT2 + 200] = W[2].T
    wk16a[0, OFF_ONES:OFF_ONES + Q] = 1.0
    gx = (G0 + DLT * np.arange(G)).astype(np.float32)
    gx16 = gx.astype(np.float16).astype(np.float32)
    wk16a[0, OFF_GX:OFF_GX + G] = gx16
    wk16a[0, OFF_XSQ:OFF_XSQ + G] = gx16 * gx16 - 1.0

    wk16b = np.zeros((128, C16B), np.float32)
    for l, off in ((3, OFF_WT3), (4, OFF_WT4)):
        fi, fo = LAYERS[l], LAYERS[l + 1]
        for ki, (ko, ks) in enumerate(_chunks(fi)):
            wk16b[0:ks, off + ki * fo:off + (ki + 1) * fo] = \
                W[l].T[ko:ko + ks, :]
    wk16b[0:128, OFF_WT5:OFF_WT5 + Q] = W[5].T[0:128, :]
    wk16b[0:72, OFF_WT5 + Q:OFF_WT5 + 2 * Q] = W[5].T[128:200, :]
    wk16b[96, OFF_WT5 + Q:OFF_WT5 + 2 * Q] = b[5]
    cg = DT * FS / CS
    wk16b[0:Q, OFF_G1:OFF_G1 + Q] = cg * A.T
    wk16b[0:Q, OFF_G1 + Q] = cg * bvec[0]

    wk32 = np.zeros((128, C32), np.float32)
    wk32[0:20, O32_W0] = W[0][:, 0]
    wk32[0:20, O32_B0] = b[0]
    wk32[0:50, O32_B1] = b[1]
    for l, off in ((2, O32_B2), (3, O32_B3), (4, O32_B4)):
        for mi, (mo, ms) in enumerate(_chunks(LAYERS[l + 1])):
            wk32[0:ms, off + mi] = b[l][mo:mo + ms]

    common = {"wk16a": wk16a.astype(np.float16),
              "wk16b": wk16b.astype(np.float16), "wk32": wk32}

    xf = np.asarray(x, np.float64).reshape(-1)
    s = (xf - G0) / DLT
    iv = np.clip(np.floor(s).astype(np.int64), 1, G - 3)
    t = s - iv
    w4 = np.stack([-t * (t - 1) * (t - 2) / 6.0,
                   (t + 1) * (t - 1) * (t - 2) / 2.0,
                   -(t + 1) * t * (t - 2) / 2.0,
                   (t + 1) * t * (t - 1) / 6.0], axis=0)  # (4, N)
    M = np.zeros((G, N_TOTAL), np.float32)
    cols = np.arange(N_TOTAL)
    for j in range(4):
        M[iv + j - 1, cols] = w4[j]
    M = M.astype(np.float16)
    shards = [{"msb": M[:, c * NC:(c + 1) * NC]} for c in range(N_CORES)]
    return common, shards


def postproc(u0d):
    """(Q+1, NC) fp16 device output -> (U0, U1) fp32 (NC, Q)."""
    a = u0d.astype(np.float32)
    U0 = a[0:Q].T * CS
    U1 = (a[0:Q] - a[Q:Q + 1]).T * CS
    return U0, U1


_NC_CACHE = None


def kernel(W0, b0, W1, b1, W2, b2, W3, b3, W4, b4, W5, b5, x, A, bvec):
    global _NC_CACHE
    W = [np.asarray(w, np.float32) for w in (W0, W1, W2, W3, W4, W5)]
    bs = [np.asarray(v, np.float32) for v in (b0, b1, b2, b3, b4, b5)]
    x = np.asarray(x, np.float32)
    A = np.asarray(A, np.float32)
    bvec = np.asarray(bvec, np.float32)

    if _NC_CACHE is None:
        _NC_CACHE = build_kernel()
    nc = _NC_CACHE

    common, shards = prep_inputs(W, bs, x, A, bvec)
    in_maps = [{**common, **shards[c]} for c in range(N_CORES)]

    from concourse.bass_utils import run_bass_kernel_spmd
    res = run_bass_kernel_spmd(nc, in_maps, list(range(N_CORES)))
    parts = [postproc(res.results[c]["U0d"]) for c in range(N_CORES)]
    U0 = np.concatenate([p[0] for p in parts], 0)
    U1 = np.concatenate([p[1] for p in parts], 0)
    return U0, U1


# revision 12
# speedup vs baseline: 1.4805x; 1.1487x over previous
"""PINN (IRK tanh-MLP + u_xx) Trainium2 kernel — grid-interpolation form.

Every activation of this network is a smooth function of the single scalar
input x, so the map x -> (U0, U1) rows is 100 smooth 1-D functions.  The
device evaluates the MLP once on a fixed 64-node uniform grid covering
[-5.5, 5.33], forms F = -(5u - 5u^3 + 5e-4*u_xx) at the nodes (u_xx via an
exact-cancellation 3-point FD in fp32), folds the IRK matrix A into a
64x101 node "combo" matrix  C = [u/CS + (DT*A.T/CS) @ F ; (DT/CS)*bvec @ F]
with one tiny matmul, and produces all outputs for the core's 8192
collocation points with a single fp16 matmul  C^T @ M,  where M is the
host-built (data-layout-only) matrix of cubic-Lagrange interpolation
weights: 4 nonzeros per column, dense (64 x 8192) fp16.  Row 100 of the
result is d = DT*(F @ bvec.T);  U0 = rows 0:100,  U1 = U0 - d (host-side
subtract of the broadcast row, as in the reference).  Cubic interpolation
on this grid reproduces the exact network outputs to ~1e-5; fp16 rounding
sets the end-to-end error at ~1e-3, well inside the 2e-2 gate.
Data-parallel over 8 cores (x batch-sharded, weights replicated).
Power-of-2 scales (FS=256 on F, CS=8 on C) keep fp16 in range; the host
multiplies outputs by CS.

Schedule notes: tanh table preloaded at t=0; layer biases are folded into
the weight packs as extra contraction rows (constant-1 rows parked in the
32-aligned gap partitions of each activation tile), so the tanh Act ops
carry no bias and L3's three full chunks merge into one Act; constants
arrive early-layers-first, the interpolation matrix in two halves behind
them; the 16-tile main loop is one matmul + one PSUM->SBUF fp16 cast per
tile (casts rotate Pool/Act/DVE); outputs leave in 5 staggered group DMAs
on the SP queue.
"""

import sys

sys.path.insert(0, "/opt/trn_rl_repo")

import numpy as np

import concourse.bass as bass
import concourse.mybir as mybir
import concourse.tile as tile
from concourse import bacc
from concourse.masks import make_identity

F32 = mybir.dt.float32
FP16 = mybir.dt.float16
AF = mybir.ActivationFunctionType
ALU = mybir.AluOpType

N_CORES = 8
N_TOTAL = 65536
NC = N_TOTAL // N_CORES  # 8192 points per core
TILE = 512
T = NC // TILE           # 16 tiles
Q = 100
DT = 0.8
LAYERS = [1, 20, 50, 200, 500, 200, Q]

G = 64                   # grid nodes
G0 = -5.5
DLT = 11.0 / 64.0        # grid spacing; nodes exactly representable in fp16
FDC = 1e-4 / (DLT * DLT)
FS = 256.0               # F-node scale (keeps u^3 inside fp16 range)
CS = 8.0                 # combo scale (outputs are U/CS; host multiplies back)

# wk16a: early constants (layer 0-2 weights + broadcast rows)
OFF_WT1 = 0                    # [128, 50]   rows 0:20 = W1.T, row 32 = b1
OFF_WT2 = OFF_WT1 + 50         # [128, 200]  rows 0:50 = W2.T, row 64 = b2
OFF_ONES = OFF_WT2 + 200       # [128, 100]  row 0 = 1.0
OFF_GX = OFF_ONES + 100        # [128, 64]   row 0 = grid x (fp16-exact)
OFF_XSQ = OFF_GX + G           # [128, 64]   row 0 = gx^2 - 1
C16A = OFF_XSQ + G
# wk16b: late constants (layer 3-5 weights + IRK combo with bvec row)
OFF_WT3 = 0                    # [128, 1000] chunk1 row 96 = b3
OFF_WT4 = OFF_WT3 + 1000       # [128, 1000] 4 k-chunks + bias chunk (row 0)
OFF_WT5 = OFF_WT4 + 1000       # [128, 200]  chunk1 row 96 = b5
OFF_G1 = OFF_WT5 + 200         # [128, 101]  rows 0:100; col 100 = bvec row
C16B = OFF_G1 + Q + 1

# wk32: layer-0 per-neuron scale/bias for the Act trick
O32_W0 = 0
O32_B0 = 1
C32 = 2

# output DMA groups (in tiles): staggered, small final groups for short tail
GROUPS = [(0, 4), (4, 4), (8, 4), (12, 2), (14, 2)]


def build_kernel(reps=1):
    nc = bacc.Bacc("TRN2", target_bir_lowering=False, debug=False,
                   num_devices=N_CORES)

    wk16a_e = nc.declare_dram_parameter("wk16a", [128, C16A], FP16,
                                        isOutput=False)
    wk16b_e = nc.declare_dram_parameter("wk16b", [128, C16B], FP16,
                                        isOutput=False)
    wk32_e = nc.declare_dram_parameter("wk32", [128, C32], F32,
                                       isOutput=False)
    msb_e = nc.declare_dram_parameter("msb", [G, NC], FP16, isOutput=False)
    u0d_e = nc.declare_dram_parameter("U0d", [Q + 1, NC], FP16,
                                      isOutput=True)

    from contextlib import ExitStack
    with tile.TileContext(nc) as tc, ExitStack() as es:
        wpool = es.enter_context(tc.tile_pool(name="weights", bufs=1))
        npool = es.enter_context(tc.tile_pool(name="nodes", bufs=1))
        pgrid = es.enter_context(tc.tile_pool(name="pgrid", bufs=2,
                                              space="PSUM"))
        pmain = es.enter_context(tc.tile_pool(name="pmain", bufs=3,
                                              space="PSUM"))

        # ---- t=0: preload tanh activation table (off critical path) -----
        scr = npool.tile([1, 2], F32, name="scr")
        nc.vector.memset(scr[0:1, 0:1], 0.0)
        nc.scalar.activation(scr[0:1, 1:2], scr[0:1, 0:1], AF.Tanh)

        # identity for the PE transpose — BEFORE the DMAs in the Pool queue
        identh = wpool.tile([128, 128], FP16, name="identh")
        make_identity(nc, identh[:, :])

        # ---- input DMAs (gpsimd/Pool queue, earliest-needed first) ------
        wk16a = wpool.tile([128, C16A], FP16, name="wk16a_sb")
        nc.gpsimd.dma_start(out=wk16a[:, :], in_=wk16a_e[:, :])
        wk32 = wpool.tile([128, C32], F32, name="wk32_sb")
        nc.gpsimd.dma_start(out=wk32[:, :], in_=wk32_e[:, :])
        wk16b = wpool.tile([128, C16B], FP16, name="wk16b_sb")
        nc.gpsimd.dma_start(out=wk16b[:, :], in_=wk16b_e[:, :])
        msb = wpool.tile([G, NC], FP16, name="msb_sb")
        HALF = NC // 2
        nc.gpsimd.dma_start(out=msb[:, 0:HALF], in_=msb_e[:, 0:HALF])
        nc.gpsimd.dma_start(out=msb[:, HALF:NC], in_=msb_e[:, HALF:NC])

        # ---- activation tiles with bias-rows pre-seeded -----------------
        # gap partitions between a layer's data rows and its constant-1 row
        # are zeroed so the (zero-padded) weight rows contract to zero.
        h0 = npool.tile([128, G], FP16, name="h0")
        nc.vector.memset(h0[0:64, :], 0.0)       # rows 20:32 gap, 33:64 pad
        nc.vector.memset(h0[32:33, :], 1.0)      # b1 row
        h1 = npool.tile([128, G], FP16, name="h1")
        nc.vector.memset(h1[32:64, :], 0.0)      # rows 50:64 gap
        nc.vector.memset(h1[64:96, :], 0.0)
        nc.vector.memset(h1[64:65, :], 1.0)      # b2 row
        h2 = npool.tile([128, 2 * G], FP16, name="h2")
        nc.vector.memset(h2[64:128, G:2 * G], 0.0)   # chunk1 rows 72:96 gap
        nc.vector.memset(h2[96:97, G:2 * G], 1.0)    # b3 row
        h3 = npool.tile([128, 5 * G], FP16, name="h3")
        nc.vector.memset(h3[0:1, 4 * G:5 * G], 1.0)  # b4 row (own k-chunk)
        h4 = npool.tile([128, 2 * G], FP16, name="h4")
        nc.vector.memset(h4[64:128, G:2 * G], 0.0)   # chunk1 rows 72:96 gap
        nc.vector.memset(h4[96:97, G:2 * G], 1.0)    # b5 row

        # ---- grid MLP eval (batch = 64 grid nodes, feature-major) -------
        ph0 = pgrid.tile([128, G], F32, name="ph0", tag="pg")
        nc.tensor.matmul(ph0[0:20, :], wk16a[0:1, OFF_ONES:OFF_ONES + 20],
                         wk16a[0:1, OFF_GX:OFF_GX + G], start=True, stop=True)
        # broadcast (gx^2 - 1) along partitions (needs only wk16a)
        pxsq = pgrid.tile([128, G], F32, name="pxsq", tag="px", bufs=1)
        nc.tensor.matmul(pxsq[0:Q, :], wk16a[0:1, OFF_ONES:OFF_ONES + Q],
                         wk16a[0:1, OFF_XSQ:OFF_XSQ + G], start=True,
                         stop=True)
        nc.scalar.activation(h0[0:20, :], ph0[0:20, :], AF.Tanh,
                             bias=wk32[0:20, O32_B0:O32_B0 + 1],
                             scale=wk32[0:20, O32_W0:O32_W0 + 1])

        # L1: 20(+b row 32) -> 50
        ph1 = pgrid.tile([128, G], F32, name="ph1", tag="pg")
        nc.tensor.matmul(ph1[0:50, :], wk16a[0:33, OFF_WT1:OFF_WT1 + 50],
                         h0[0:33, :], start=True, stop=True)
        nc.scalar.activation(h1[0:50, :], ph1[0:50, :], AF.Tanh)

        # L2: 50(+b row 64) -> 200 (chunks 128 + 72)
        ph2 = pgrid.tile([128, 2 * G], F32, name="ph2", tag="pg")
        nc.tensor.matmul(ph2[0:128, 0:G], wk16a[0:65, OFF_WT2:OFF_WT2 + 128],
                         h1[0:65, :], start=True, stop=True)
        nc.tensor.matmul(ph2[0:72, G:2 * G],
                         wk16a[0:65, OFF_WT2 + 128:OFF_WT2 + 200],
                         h1[0:65, :], start=True, stop=True)
        nc.scalar.activation(h2[0:128, 0:G], ph2[0:128, 0:G], AF.Tanh)
        nc.scalar.activation(h2[0:72, G:2 * G], ph2[0:72, G:2 * G], AF.Tanh)

        # L3: 200 (chunks 128 + 72(+b row 96)) -> 500 (4 chunks)
        ph3a = pgrid.tile([128, 3 * G], F32, name="ph3a", tag="pg")
        ph3b = pgrid.tile([128, G], F32, name="ph3b", tag="pg")
        for mi in range(4):
            dst = ph3a[0:128, mi * G:(mi + 1) * G] if mi < 3 else \
                ph3b[0:116, 0:G]
            nc.tensor.matmul(dst,
                             wk16b[0:128, OFF_WT3 + mi * 128:
                                   OFF_WT3 + mi * 128 + (128 if mi < 3
                                                         else 116)],
                             h2[0:128, 0:G], start=True, stop=False)
            nc.tensor.matmul(dst,
                             wk16b[0:97, OFF_WT3 + 500 + mi * 128:
                                   OFF_WT3 + 500 + mi * 128 + (128 if mi < 3
                                                               else 116)],
                             h2[0:97, G:2 * G], start=False, stop=True)
        nc.scalar.activation(h3[0:128, 0:3 * G], ph3a[0:128, :], AF.Tanh)
        nc.scalar.activation(h3[0:116, 3 * G:4 * G], ph3b[0:116, :], AF.Tanh)

        # L4: 500 (4 chunks) + b chunk (h3 row 0 of block 4) -> 200
        ph4 = pgrid.tile([128, 2 * G], F32, name="ph4", tag="pg")
        for mi, ms in ((0, 128), (1, 72)):
            dst = ph4[0:ms, mi * G:(mi + 1) * G]
            for ki in range(5):
                ks = (128, 128, 128, 116, 1)[ki]
                nc.tensor.matmul(dst,
                                 wk16b[0:ks, OFF_WT4 + ki * 200 + mi * 128:
                                       OFF_WT4 + ki * 200 + mi * 128 + ms],
                                 h3[0:ks, ki * G:(ki + 1) * G],
                                 start=(ki == 0), stop=(ki == 4))
        nc.scalar.activation(h4[0:128, 0:G], ph4[0:128, 0:G], AF.Tanh)
        nc.scalar.activation(h4[0:72, G:2 * G], ph4[0:72, G:2 * G], AF.Tanh)

        # L5: 200 (chunks 128 + 72(+b5 row 96)) -> (100, G)
        pL5 = pgrid.tile([128, G], F32, name="pL5", tag="pg")
        nc.tensor.matmul(pL5[0:Q, :], wk16b[0:128, OFF_WT5:OFF_WT5 + Q],
                         h4[0:128, 0:G], start=True, stop=False)
        nc.tensor.matmul(pL5[0:Q, :],
                         wk16b[0:97, OFF_WT5 + Q:OFF_WT5 + 2 * Q],
                         h4[0:97, G:2 * G], start=False, stop=True)

        # ---- node-side math (all [100, 64] fp32, trivial sizes) ---------
        # u = pxsq * pL5 - 1     (rows 96:128 zeroed so combo row 100 = d)
        u = npool.tile([128, G], F32, name="u_fm")
        nc.vector.memset(u[96:128, :], 0.0)
        nc.vector.tensor_mul(u[0:Q, :], pxsq[0:Q, :], pL5[0:Q, :])
        nc.vector.tensor_scalar_add(u[0:Q, :], u[0:Q, :], -1.0)

        # wfd = u[i-1] + u[i+1] - 2 u[i]  (grid-axis FD; edge cols zero)
        wfd = npool.tile([128, G], F32, name="wfd")
        nc.vector.memset(wfd[0:Q, 0:1], 0.0)
        nc.vector.memset(wfd[0:Q, G - 1:G], 0.0)
        z = npool.tile([128, G], F32, name="z")
        nc.vector.tensor_add(z[0:Q, 1:G - 1], u[0:Q, 0:G - 2], u[0:Q, 2:G])
        nc.vector.scalar_tensor_tensor(wfd[0:Q, 1:G - 1], u[0:Q, 1:G - 1],
                                       -2.0, z[0:Q, 1:G - 1], ALU.mult,
                                       ALU.add)

        # Fn = (5/FS)*(u^3 - u) - (5*FDC/FS)*wfd
        usq = npool.tile([128, G], F32, name="usq")
        nc.vector.tensor_mul(usq[0:Q, :], u[0:Q, :], u[0:Q, :])
        nc.vector.tensor_scalar_add(usq[0:Q, :], usq[0:Q, :], -1.0)
        gs = npool.tile([128, G], F32, name="gs")
        nc.vector.scalar_tensor_tensor(gs[0:Q, :], u[0:Q, :], 5.0 / FS,
                                       usq[0:Q, :], ALU.mult, ALU.mult)
        fn16 = npool.tile([128, G], FP16, name="fn16")
        nc.vector.scalar_tensor_tensor(fn16[0:Q, :], wfd[0:Q, :],
                                       -5.0 * FDC / FS, gs[0:Q, :], ALU.mult,
                                       ALU.add)

        # ---- combo: C[0:100] = u/CS + G1' @ Fn ; C[100] = bvec' @ Fn ----
        pc = pgrid.tile([128, G], F32, name="pc", tag="pg")
        nc.tensor.matmul(pc[0:Q + 1, :], wk16b[0:Q, OFF_G1:OFF_G1 + Q + 1],
                         fn16[0:Q, :], start=True, stop=True)
        c16 = npool.tile([128, G], FP16, name="c16")
        nc.vector.scalar_tensor_tensor(c16[0:Q + 1, :], u[0:Q + 1, :],
                                       1.0 / CS, pc[0:Q + 1, :], ALU.mult,
                                       ALU.add)
        ptr = pgrid.tile([128, 128], FP16, name="ptr", tag="pt", bufs=1)
        nc.tensor.transpose(ptr[0:G, 0:Q + 1], c16[0:Q + 1, 0:G],
                            identh[0:Q + 1, 0:Q + 1])
        lt = npool.tile([G, 128], FP16, name="lt")
        nc.vector.memset(lt[:, 96:128], 0.0)
        nc.vector.tensor_copy(lt[:, 0:Q + 1], ptr[0:G, 0:Q + 1])

        # ---- main interpolation loop: 1 matmul + 1 cast per tile --------
        casters = [nc.gpsimd.tensor_copy, nc.scalar.copy,
                   nc.vector.tensor_copy]
        ou = wpool.tile([128, NC], FP16, name="ou")
        for _rep in range(reps):
            for t in range(T):
                sl = slice(t * TILE, (t + 1) * TILE)
                pa = pmain.tile([128, TILE], F32, name=f"pa{t}", tag="pa")
                nc.tensor.matmul(pa[:, :], lt[0:G, 0:128], msb[0:G, sl],
                                 start=True, stop=True)
                casters[t % 3](ou[0:Q + 1, sl], pa[0:Q + 1, :])
                for g0t, gn in GROUPS:
                    if t == g0t + gn - 1:
                        gs_ = slice(g0t * TILE, (g0t + gn) * TILE)
                        nc.sync.dma_start(out=u0d_e[0:Q + 1, gs_],
                                          in_=ou[0:Q + 1, gs_])

    nc.compile()
    return nc


def prep_inputs(W, b, x, A, bvec):
    """Host-side prep: packed replicated constants + per-core M matrices."""
    wk16a = np.zeros((128, C16A), np.float32)
    wk16a[0:20, OFF_WT1:OFF_WT1 + 50] = W[1].T
    wk16a[32, OFF_WT1:OFF_WT1 + 50] = b[1]
    wk16a[0:50, OFF_WT2:OFF_WT2 + 200] = W[2].T
    wk16a[64, OFF_WT2:OFF_WT2 + 200] = b[2]
    wk16a[0, OFF_ONES:OFF_ONES + Q] = 1.0
    gx = (G0 + DLT * np.arange(G)).astype(np.float32)
    gx16 = gx.astype(np.float16).astype(np.float32)
    wk16a[0, OFF_GX:OFF_GX + G] = gx16
    wk16a[0, OFF_XSQ:OFF_XSQ + G] = gx16 * gx16 - 1.0

    wk16b = np.zeros((128, C16B), np.float32)
    wk16b[0:128, OFF_WT3:OFF_WT3 + 500] = W[3].T[0:128, :]
    wk16b[0:72, OFF_WT3 + 500:OFF_WT3 + 1000] = W[3].T[128:200, :]
    wk16b[96, OFF_WT3 + 500:OFF_WT3 + 1000] = b[3]
    for ki, (ko, ks) in enumerate(((0, 128), (128, 128), (256, 128),
                                   (384, 116))):
        wk16b[0:ks, OFF_WT4 + ki * 200:OFF_WT4 + (ki + 1) * 200] = \
            W[4].T[ko:ko + ks, :]
    wk16b[0, OFF_WT4 + 800:OFF_WT4 + 1000] = b[4]
    wk16b[0:128, OFF_WT5:OFF_WT5 + Q] = W[5].T[0:128, :]
    wk16b[0:72, OFF_WT5 + Q:OFF_WT5 + 2 * Q] = W[5].T[128:200, :]
    wk16b[96, OFF_WT5 + Q:OFF_WT5 + 2 * Q] = b[5]
    cg = DT * FS / CS
    wk16b[0:Q, OFF_G1:OFF_G1 + Q] = cg * A.T
    wk16b[0:Q, OFF_G1 + Q] = cg * bvec[0]

    wk32 = np.zeros((128, C32), np.float32)
    wk32[0:20, O32_W0] = W[0][:, 0]
    wk32[0:20, O32_B0] = b[0]

    common = {"wk16a": wk16a.astype(np.float16),
              "wk16b": wk16b.astype(np.float16), "wk32": wk32}

    xf = np.asarray(x, np.float64).reshape(-1)
    s = (xf - G0) / DLT
    iv = np.clip(np.floor(s).astype(np.int64), 1, G - 3)
    t = s - iv
    w4 = np.stack([-t * (t - 1) * (t - 2) / 6.0,
                   (t + 1) * (t - 1) * (t - 2) / 2.0,
                   -(t + 1) * t * (t - 2) / 2.0,
                   (t + 1) * t * (t - 1) / 6.0], axis=0)  # (4, N)
    M = np.zeros((G, N_TOTAL), np.float32)
    cols = np.arange(N_TOTAL)
    for j in range(4):
        M[iv + j - 1, cols] = w4[j]
    M = M.astype(np.float16)
    shards = [{"msb": M[:, c * NC:(c + 1) * NC]} for c in range(N_CORES)]
    return common, shards


def postproc(u0d):
    """(Q+1, NC) fp16 device output -> (U0, U1) fp32 (NC, Q)."""
    a = u0d.astype(np.float32)
    U0 = a[0:Q].T * CS
    U1 = (a[0:Q] - a[Q:Q + 1]).T * CS
    return U0, U1


_NC_CACHE = None


def kernel(W0, b0, W1, b1, W2, b2, W3, b3, W4, b4, W5, b5, x, A, bvec):
    global _NC_CACHE
    W = [np.asarray(w, np.float32) for w in (W0, W1, W2, W3, W4, W5)]
    bs = [np.asarray(v, np.float32) for v in (b0, b1, b2, b3, b4, b5)]
    x = np.asarray(x, np.float32)
    A = np.asarray(A, np.float32)
    bvec = np.asarray(bvec, np.float32)

    if _NC_CACHE is None:
        _NC_CACHE = build_kernel()
    nc = _NC_CACHE

    common, shards = prep_inputs(W, bs, x, A, bvec)
    in_maps = [{**common, **shards[c]} for c in range(N_CORES)]

    from concourse.bass_utils import run_bass_kernel_spmd
    res = run_bass_kernel_spmd(nc, in_maps, list(range(N_CORES)))
    parts = [postproc(res.results[c]["U0d"]) for c in range(N_CORES)]
    U0 = np.concatenate([p[0] for p in parts], 0)
    U1 = np.concatenate([p[1] for p in parts], 0)
    return U0, U1


# revision 13
# speedup vs baseline: 1.5197x; 1.0264x over previous
"""PINN (IRK tanh-MLP + u_xx) Trainium2 kernel — grid-interpolation form.

Every activation of this network is a smooth function of the single scalar
input x, so the map x -> (U0, U1) rows is 100 smooth 1-D functions.  The
device evaluates the MLP once on a fixed 64-node uniform grid covering
[-5.5, 5.33], forms F = -(5u - 5u^3 + 5e-4*u_xx) at the nodes (u_xx via an
exact-cancellation 3-point FD in fp32), folds the IRK matrix A into a
64x101 node "combo" matrix  C = [u/CS + (DT*A.T/CS) @ F ; (DT/CS)*bvec @ F]
with one tiny matmul, and produces all outputs for the core's 8192
collocation points with a single fp16 matmul  C^T @ M,  where M is the
host-built (data-layout-only) matrix of cubic-Lagrange interpolation
weights: 4 nonzeros per column, dense (64 x 8192) fp16.  Row 100 of the
result is d = DT*(F @ bvec.T);  U0 = rows 0:100,  U1 = U0 - d (host-side
subtract of the broadcast row, as in the reference).  Cubic interpolation
on this grid reproduces the exact network outputs to ~1e-5; fp16 rounding
sets the end-to-end error at ~1e-3, well inside the 2e-2 gate.
Data-parallel over 8 cores (x batch-sharded, weights replicated).
Power-of-2 scales (FS=256 on F, CS=8 on C) keep fp16 in range; the host
multiplies outputs by CS.

Schedule notes: tanh table preloaded at t=0; layer biases are folded into
the weight packs as extra contraction rows (constant-1 rows parked in the
32-aligned gap partitions of each activation tile), so the tanh Act ops
carry no bias and L3's three full chunks merge into one Act; constants
arrive early-layers-first, the interpolation matrix in two halves behind
them; the 16-tile main loop is one matmul + one PSUM->SBUF fp16 cast per
tile (casts rotate Pool/Act/DVE); outputs leave in 5 staggered group DMAs
on the SP queue.
"""

import sys

sys.path.insert(0, "/opt/trn_rl_repo")

import numpy as np

import concourse.bass as bass
import concourse.mybir as mybir
import concourse.tile as tile
from concourse import bacc
from concourse.masks import make_identity

F32 = mybir.dt.float32
FP16 = mybir.dt.float16
AF = mybir.ActivationFunctionType
ALU = mybir.AluOpType

N_CORES = 8
N_TOTAL = 65536
NC = N_TOTAL // N_CORES  # 8192 points per core
TILE = 512
T = NC // TILE           # 16 tiles
Q = 100
DT = 0.8
LAYERS = [1, 20, 50, 200, 500, 200, Q]

G = 64                   # grid nodes
G0 = -5.5
DLT = 11.0 / 64.0        # grid spacing; nodes exactly representable in fp16
FDC = 1e-4 / (DLT * DLT)
FS = 256.0               # F-node scale (keeps u^3 inside fp16 range)
CS = 8.0                 # combo scale (outputs are U/CS; host multiplies back)

# wk16a: early constants (layer 0-2 weights + broadcast rows)
OFF_WT1 = 0                    # [128, 50]   rows 0:20 = W1.T, row 32 = b1
OFF_WT2 = OFF_WT1 + 50         # [128, 200]  rows 0:50 = W2.T, row 64 = b2
OFF_ONES = OFF_WT2 + 200       # [128, 100]  row 0 = 1.0
OFF_GX = OFF_ONES + 100        # [128, 64]   row 0 = grid x (fp16-exact)
OFF_XSQ = OFF_GX + G           # [128, 64]   row 0 = gx^2 - 1
OFF_WB0 = OFF_XSQ + G          # [128, 4]    fp32 w0c/b0c as fp16 byte pairs
C16A = OFF_WB0 + 4
# wk16b: late constants (layer 3-5 weights + IRK combo with bvec row)
OFF_WT3 = 0                    # [128, 1000] chunk1 row 96 = b3
OFF_WT4 = OFF_WT3 + 1000       # [128, 1000] 4 k-chunks + bias chunk (row 0)
OFF_WT5 = OFF_WT4 + 1000       # [128, 200]  chunk1 row 96 = b5
OFF_G1 = OFF_WT5 + 200         # [128, 101]  rows 0:100; col 100 = bvec row
C16B = OFF_G1 + Q + 1

# output DMA groups (in tiles): staggered, small final groups for short tail
GROUPS = [(0, 4), (4, 4), (8, 4), (12, 2), (14, 2)]


def build_kernel(reps=1):
    nc = bacc.Bacc("TRN2", target_bir_lowering=False, debug=False,
                   num_devices=N_CORES)

    wk16a_e = nc.declare_dram_parameter("wk16a", [128, C16A], FP16,
                                        isOutput=False)
    wk16b_e = nc.declare_dram_parameter("wk16b", [128, C16B], FP16,
                                        isOutput=False)
    msb_e = nc.declare_dram_parameter("msb", [G, NC], FP16, isOutput=False)
    u0d_e = nc.declare_dram_parameter("U0d", [Q + 1, NC], FP16,
                                      isOutput=True)

    from contextlib import ExitStack
    with tile.TileContext(nc) as tc, ExitStack() as es:
        wpool = es.enter_context(tc.tile_pool(name="weights", bufs=1))
        npool = es.enter_context(tc.tile_pool(name="nodes", bufs=1))
        pgrid = es.enter_context(tc.tile_pool(name="pgrid", bufs=2,
                                              space="PSUM"))
        pmain = es.enter_context(tc.tile_pool(name="pmain", bufs=3,
                                              space="PSUM"))

        # ---- t=0: preload tanh activation table (off critical path) -----
        scr = npool.tile([1, 2], F32, name="scr")
        nc.vector.memset(scr[0:1, 0:1], 0.0)
        nc.scalar.activation(scr[0:1, 1:2], scr[0:1, 0:1], AF.Tanh)

        # identity for the PE transpose — BEFORE the DMAs in the Pool queue
        identh = wpool.tile([128, 128], FP16, name="identh")
        make_identity(nc, identh[:, :])

        # ---- input DMAs (gpsimd/Pool queue, earliest-needed first) ------
        wk16a = wpool.tile([128, C16A], FP16, name="wk16a_sb")
        nc.gpsimd.dma_start(out=wk16a[:, :], in_=wk16a_e[:, :])
        wk16b = wpool.tile([128, C16B], FP16, name="wk16b_sb")
        nc.gpsimd.dma_start(out=wk16b[:, :], in_=wk16b_e[:, :])
        HALF = NC // 2
        msb0 = wpool.tile([G, HALF], FP16, name="msb0_sb")
        nc.gpsimd.dma_start(out=msb0[:, :], in_=msb_e[:, 0:HALF])
        msb1 = wpool.tile([G, HALF], FP16, name="msb1_sb")
        nc.gpsimd.dma_start(out=msb1[:, :], in_=msb_e[:, HALF:NC])

        # ---- activation tiles with bias-rows pre-seeded -----------------
        # gap partitions between a layer's data rows and its constant-1 row
        # are zeroed so the (zero-padded) weight rows contract to zero.
        h0 = npool.tile([128, G], FP16, name="h0")
        nc.vector.memset(h0[0:64, :], 0.0)       # rows 20:32 gap, 33:64 pad
        nc.vector.memset(h0[32:33, :], 1.0)      # b1 row
        h1 = npool.tile([128, G], FP16, name="h1")
        nc.vector.memset(h1[32:64, :], 0.0)      # rows 50:64 gap
        nc.vector.memset(h1[64:96, :], 0.0)
        nc.vector.memset(h1[64:65, :], 1.0)      # b2 row
        h2 = npool.tile([128, 2 * G], FP16, name="h2")
        nc.vector.memset(h2[64:128, G:2 * G], 0.0)   # chunk1 rows 72:96 gap
        nc.vector.memset(h2[96:97, G:2 * G], 1.0)    # b3 row
        h3 = npool.tile([128, 5 * G], FP16, name="h3")
        nc.vector.memset(h3[0:1, 4 * G:5 * G], 1.0)  # b4 row (own k-chunk)
        h4 = npool.tile([128, 2 * G], FP16, name="h4")
        nc.vector.memset(h4[64:128, G:2 * G], 0.0)   # chunk1 rows 72:96 gap
        nc.vector.memset(h4[96:97, G:2 * G], 1.0)    # b5 row

        # ---- grid MLP eval (batch = 64 grid nodes, feature-major) -------
        ph0 = pgrid.tile([128, G], F32, name="ph0", tag="pg")
        nc.tensor.matmul(ph0[0:20, :], wk16a[0:1, OFF_ONES:OFF_ONES + 20],
                         wk16a[0:1, OFF_GX:OFF_GX + G], start=True, stop=True)
        # broadcast (gx^2 - 1) along partitions (needs only wk16a)
        pxsq = pgrid.tile([128, G], F32, name="pxsq", tag="px", bufs=1)
        nc.tensor.matmul(pxsq[0:Q, :], wk16a[0:1, OFF_ONES:OFF_ONES + Q],
                         wk16a[0:1, OFF_XSQ:OFF_XSQ + G], start=True,
                         stop=True)
        w0ap = wk16a[0:20, OFF_WB0:OFF_WB0 + 2].bitcast(F32)
        b0ap = wk16a[0:20, OFF_WB0 + 2:OFF_WB0 + 4].bitcast(F32)
        nc.scalar.activation(h0[0:20, :], ph0[0:20, :], AF.Tanh,
                             bias=b0ap, scale=w0ap)

        # L1: 20(+b row 32) -> 50
        ph1 = pgrid.tile([128, G], F32, name="ph1", tag="pg")
        nc.tensor.matmul(ph1[0:50, :], wk16a[0:33, OFF_WT1:OFF_WT1 + 50],
                         h0[0:33, :], start=True, stop=True)
        nc.scalar.activation(h1[0:50, :], ph1[0:50, :], AF.Tanh)

        # L2: 50(+b row 64) -> 200 (chunks 128 + 72)
        ph2 = pgrid.tile([128, 2 * G], F32, name="ph2", tag="pg")
        nc.tensor.matmul(ph2[0:128, 0:G], wk16a[0:65, OFF_WT2:OFF_WT2 + 128],
                         h1[0:65, :], start=True, stop=True)
        nc.tensor.matmul(ph2[0:72, G:2 * G],
                         wk16a[0:65, OFF_WT2 + 128:OFF_WT2 + 200],
                         h1[0:65, :], start=True, stop=True)
        nc.scalar.activation(h2[0:128, 0:G], ph2[0:128, 0:G], AF.Tanh)
        nc.scalar.activation(h2[0:72, G:2 * G], ph2[0:72, G:2 * G], AF.Tanh)

        # L3: 200 (chunks 128 + 72(+b row 96)) -> 500 (4 chunks)
        ph3a = pgrid.tile([128, 3 * G], F32, name="ph3a", tag="pg")
        ph3b = pgrid.tile([128, G], F32, name="ph3b", tag="pg")
        for mi in range(4):
            dst = ph3a[0:128, mi * G:(mi + 1) * G] if mi < 3 else \
                ph3b[0:116, 0:G]
            nc.tensor.matmul(dst,
                             wk16b[0:128, OFF_WT3 + mi * 128:
                                   OFF_WT3 + mi * 128 + (128 if mi < 3
                                                         else 116)],
                             h2[0:128, 0:G], start=True, stop=False)
            nc.tensor.matmul(dst,
                             wk16b[0:97, OFF_WT3 + 500 + mi * 128:
                                   OFF_WT3 + 500 + mi * 128 + (128 if mi < 3
                                                               else 116)],
                             h2[0:97, G:2 * G], start=False, stop=True)
        nc.scalar.activation(h3[0:128, 0:3 * G], ph3a[0:128, :], AF.Tanh)
        nc.scalar.activation(h3[0:116, 3 * G:4 * G], ph3b[0:116, :], AF.Tanh)

        # L4: 500 (4 chunks) + b chunk (h3 row 0 of block 4) -> 200
        ph4 = pgrid.tile([128, 2 * G], F32, name="ph4", tag="pg")
        for mi, ms in ((0, 128), (1, 72)):
            dst = ph4[0:ms, mi * G:(mi + 1) * G]
            for ki in range(5):
                ks = (128, 128, 128, 116, 1)[ki]
                nc.tensor.matmul(dst,
                                 wk16b[0:ks, OFF_WT4 + ki * 200 + mi * 128:
                                       OFF_WT4 + ki * 200 + mi * 128 + ms],
                                 h3[0:ks, ki * G:(ki + 1) * G],
                                 start=(ki == 0), stop=(ki == 4))
        nc.scalar.activation(h4[0:128, 0:G], ph4[0:128, 0:G], AF.Tanh)
        nc.scalar.activation(h4[0:72, G:2 * G], ph4[0:72, G:2 * G], AF.Tanh)

        # L5: 200 (chunks 128 + 72(+b5 row 96)) -> (100, G)
        pL5 = pgrid.tile([128, G], F32, name="pL5", tag="pg")
        nc.tensor.matmul(pL5[0:Q, :], wk16b[0:128, OFF_WT5:OFF_WT5 + Q],
                         h4[0:128, 0:G], start=True, stop=False)
        nc.tensor.matmul(pL5[0:Q, :],
                         wk16b[0:97, OFF_WT5 + Q:OFF_WT5 + 2 * Q],
                         h4[0:97, G:2 * G], start=False, stop=True)

        # ---- node-side math (all [100, 64] fp32, trivial sizes) ---------
        # u = pxsq * pL5 - 1     (rows 96:128 zeroed so combo row 100 = d)
        u = npool.tile([128, G], F32, name="u_fm")
        nc.vector.memset(u[96:128, :], 0.0)
        nc.vector.tensor_mul(u[0:Q, :], pxsq[0:Q, :], pL5[0:Q, :])
        nc.vector.tensor_scalar_add(u[0:Q, :], u[0:Q, :], -1.0)

        # wfd = u[i-1] + u[i+1] - 2 u[i]  (grid-axis FD; edge cols zero)
        wfd = npool.tile([128, G], F32, name="wfd")
        nc.vector.memset(wfd[0:Q, 0:1], 0.0)
        nc.vector.memset(wfd[0:Q, G - 1:G], 0.0)
        z = npool.tile([128, G], F32, name="z")
        nc.vector.tensor_add(z[0:Q, 1:G - 1], u[0:Q, 0:G - 2], u[0:Q, 2:G])
        nc.vector.scalar_tensor_tensor(wfd[0:Q, 1:G - 1], u[0:Q, 1:G - 1],
                                       -2.0, z[0:Q, 1:G - 1], ALU.mult,
                                       ALU.add)

        # Fn = (5/FS)*(u^3 - u) - (5*FDC/FS)*wfd
        usq = npool.tile([128, G], F32, name="usq")
        nc.vector.tensor_mul(usq[0:Q, :], u[0:Q, :], u[0:Q, :])
        nc.vector.tensor_scalar_add(usq[0:Q, :], usq[0:Q, :], -1.0)
        gs = npool.tile([128, G], F32, name="gs")
        nc.vector.scalar_tensor_tensor(gs[0:Q, :], u[0:Q, :], 5.0 / FS,
                                       usq[0:Q, :], ALU.mult, ALU.mult)
        fn16 = npool.tile([128, G], FP16, name="fn16")
        nc.vector.scalar_tensor_tensor(fn16[0:Q, :], wfd[0:Q, :],
                                       -5.0 * FDC / FS, gs[0:Q, :], ALU.mult,
                                       ALU.add)

        # ---- combo: C[0:100] = u/CS + G1' @ Fn ; C[100] = bvec' @ Fn ----
        pc = pgrid.tile([128, G], F32, name="pc", tag="pg")
        nc.tensor.matmul(pc[0:Q + 1, :], wk16b[0:Q, OFF_G1:OFF_G1 + Q + 1],
                         fn16[0:Q, :], start=True, stop=True)
        c16 = npool.tile([128, G], FP16, name="c16")
        nc.vector.scalar_tensor_tensor(c16[0:Q + 1, :], u[0:Q + 1, :],
                                       1.0 / CS, pc[0:Q + 1, :], ALU.mult,
                                       ALU.add)
        ptr = pgrid.tile([128, 128], FP16, name="ptr", tag="pt", bufs=1)
        nc.tensor.transpose(ptr[0:G, 0:Q + 1], c16[0:Q + 1, 0:G],
                            identh[0:Q + 1, 0:Q + 1])
        lt = npool.tile([G, 128], FP16, name="lt")
        nc.vector.memset(lt[:, 96:128], 0.0)
        nc.vector.tensor_copy(lt[:, 0:Q + 1], ptr[0:G, 0:Q + 1])

        # ---- main interpolation loop: 1 matmul + 1 cast per tile --------
        casters = [nc.gpsimd.tensor_copy, nc.scalar.copy,
                   nc.vector.tensor_copy]
        ou = wpool.tile([128, NC], FP16, name="ou")
        for _rep in range(reps):
            for t in range(T):
                sl = slice(t * TILE, (t + 1) * TILE)
                mh = msb0 if t < T // 2 else msb1
                hs = slice((t % (T // 2)) * TILE, (t % (T // 2) + 1) * TILE)
                pa = pmain.tile([128, TILE], F32, name=f"pa{t}", tag="pa")
                nc.tensor.matmul(pa[:, :], lt[0:G, 0:128], mh[0:G, hs],
                                 start=True, stop=True)
                casters[t % 3](ou[0:Q + 1, sl], pa[0:Q + 1, :])
                for g0t, gn in GROUPS:
                    if t == g0t + gn - 1:
                        gs_ = slice(g0t * TILE, (g0t + gn) * TILE)
                        nc.sync.dma_start(out=u0d_e[0:Q + 1, gs_],
                                          in_=ou[0:Q + 1, gs_])

    nc.compile()
    return nc


def prep_inputs(W, b, x, A, bvec):
    """Host-side prep: packed replicated constants + per-core M matrices."""
    wk16a = np.zeros((128, C16A), np.float32)
    wk16a[0:20, OFF_WT1:OFF_WT1 + 50] = W[1].T
    wk16a[32, OFF_WT1:OFF_WT1 + 50] = b[1]
    wk16a[0:50, OFF_WT2:OFF_WT2 + 200] = W[2].T
    wk16a[64, OFF_WT2:OFF_WT2 + 200] = b[2]
    wk16a[0, OFF_ONES:OFF_ONES + Q] = 1.0
    gx = (G0 + DLT * np.arange(G)).astype(np.float32)
    gx16 = gx.astype(np.float16).astype(np.float32)
    wk16a[0, OFF_GX:OFF_GX + G] = gx16
    wk16a[0, OFF_XSQ:OFF_XSQ + G] = gx16 * gx16 - 1.0
    wk16a16 = wk16a.astype(np.float16)
    w0b0 = np.zeros((128, 2), np.float32)
    w0b0[0:20, 0] = W[0][:, 0]
    w0b0[0:20, 1] = b[0]
    wk16a16[:, OFF_WB0:OFF_WB0 + 4] = w0b0.view(np.float16)

    wk16b = np.zeros((128, C16B), np.float32)
    wk16b[0:128, OFF_WT3:OFF_WT3 + 500] = W[3].T[0:128, :]
    wk16b[0:72, OFF_WT3 + 500:OFF_WT3 + 1000] = W[3].T[128:200, :]
    wk16b[96, OFF_WT3 + 500:OFF_WT3 + 1000] = b[3]
    for ki, (ko, ks) in enumerate(((0, 128), (128, 128), (256, 128),
                                   (384, 116))):
        wk16b[0:ks, OFF_WT4 + ki * 200:OFF_WT4 + (ki + 1) * 200] = \
            W[4].T[ko:ko + ks, :]
    wk16b[0, OFF_WT4 + 800:OFF_WT4 + 1000] = b[4]
    wk16b[0:128, OFF_WT5:OFF_WT5 + Q] = W[5].T[0:128, :]
    wk16b[0:72, OFF_WT5 + Q:OFF_WT5 + 2 * Q] = W[5].T[128:200, :]
    wk16b[96, OFF_WT5 + Q:OFF_WT5 + 2 * Q] = b[5]
    cg = DT * FS / CS
    wk16b[0:Q, OFF_G1:OFF_G1 + Q] = cg * A.T
    wk16b[0:Q, OFF_G1 + Q] = cg * bvec[0]

    common = {"wk16a": wk16a16,
              "wk16b": wk16b.astype(np.float16)}

    xf = np.asarray(x, np.float64).reshape(-1)
    s = (xf - G0) / DLT
    iv = np.clip(np.floor(s).astype(np.int64), 1, G - 3)
    t = s - iv
    w4 = np.stack([-t * (t - 1) * (t - 2) / 6.0,
                   (t + 1) * (t - 1) * (t - 2) / 2.0,
                   -(t + 1) * t * (t - 2) / 2.0,
                   (t + 1) * t * (t - 1) / 6.0], axis=0)  # (4, N)
    M = np.zeros((G, N_TOTAL), np.float32)
    cols = np.arange(N_TOTAL)
    for j in range(4):
        M[iv + j - 1, cols] = w4[j]
    M = M.astype(np.float16)
    shards = [{"msb": M[:, c * NC:(c + 1) * NC]} for c in range(N_CORES)]
    return common, shards


def postproc(u0d):
    """(Q+1, NC) fp16 device output -> (U0, U1) fp32 (NC, Q)."""
    a = u0d.astype(np.float32)
    U0 = a[0:Q].T * CS
    U1 = (a[0:Q] - a[Q:Q + 1]).T * CS
    return U0, U1


_NC_CACHE = None


def kernel(W0, b0, W1, b1, W2, b2, W3, b3, W4, b4, W5, b5, x, A, bvec):
    global _NC_CACHE
    W = [np.asarray(w, np.float32) for w in (W0, W1, W2, W3, W4, W5)]
    bs = [np.asarray(v, np.float32) for v in (b0, b1, b2, b3, b4, b5)]
    x = np.asarray(x, np.float32)
    A = np.asarray(A, np.float32)
    bvec = np.asarray(bvec, np.float32)

    if _NC_CACHE is None:
        _NC_CACHE = build_kernel()
    nc = _NC_CACHE

    common, shards = prep_inputs(W, bs, x, A, bvec)
    in_maps = [{**common, **shards[c]} for c in range(N_CORES)]

    from concourse.bass_utils import run_bass_kernel_spmd
    res = run_bass_kernel_spmd(nc, in_maps, list(range(N_CORES)))
    parts = [postproc(res.results[c]["U0d"]) for c in range(N_CORES)]
    U0 = np.concatenate([p[0] for p in parts], 0)
    U1 = np.concatenate([p[1] for p in parts], 0)
    return U0, U1
